# revision 17
# baseline (speedup 1.0000x reference)
"""Self-contained Trainium2 Bass kernel for nn_AttentionHead_89687507076307.

Problem: single-head causal attention, B=8, S=2048, D_IN=1024, D_OUT=64, fp32.
Sharding: data-parallel over batch -- each of the 8 NeuronCores computes one
batch element end to end; no collectives.

Host marshaling (part of input sharding): X tensors are transposed to
[D, S] layout and cast to bf16 per core; weights are cast to bf16 and split
into 128-row d-chunks.  The device then runs a pure-bf16 matmul pipeline
(PE native rate) with no on-device transposes of X:

  kT/qT/vT [64,S]: W-chunk stationary, X^T pumped  (contract d, PSUM accum)
  vaug  [k,65]   : PE transpose of vT tiles + ones col (natural [S,64]+sums)
  scoresT [k,q]  : kT-tile stationary, qT pumped    (contract e=64); two
                   off-diagonal k-tiles share one [128,1024] PSUM pair
  expT           : ACT exp(0.125 * scores) -> bf16 over the 1024-wide pair
                   (halves ACT instruction count); causal quarter-mask on
                   diagonal tiles (DVE); fully-masked columns skipped
  av [65, q]     : vaug stationary, expT pumped     (contract k; row 64=sums)

The load stream is ordered wkqv k0 q0 v0 k1 q1 v1 k2 q2 k3 q3 v2 v3 so the
last query block's scores/exp chain starts early and only the cheap
v-gated AV work trails the final DMA; a PE warmup bridge holds the HAM
clock gate at full rate until the first data lands.  Device emits
unnormalized av tiles [4, 65, 512] bf16; the host divides by the sums row
and transposes back to [S, 64] during the gather/unshard step.
"""
import sys

for _p in ("/opt/trn_rl_repo",):
    if _p not in sys.path:
        sys.path.append(_p)

from contextlib import ExitStack

import numpy as np

import concourse.bass as bass
import concourse.mybir as mybir
import concourse.tile as tile
from concourse import bacc

B, S, D, E = 8, 2048, 1024, 64
SB = 512               # q block size
NSB = S // SB          # 4
NKT = S // 128         # 16 k-tiles
NDC = D // 128         # 8 d-chunks
F32 = mybir.dt.float32
BF16 = mybir.dt.bfloat16
EXP = mybir.ActivationFunctionType.Exp
N_CORES = 8
N_WARMUP = 15

# per-sb load order of (tensor, sb) blocks; projections/attention consume in
# this order too.  k3/q3 pulled ahead of v2/v3 so the tail after the last
# load is only v-projection + diagonal AV.
LOAD_ORDER = [
    ("k", 0), ("q", 0), ("v", 0),
    ("k", 1), ("q", 1), ("v", 1),
    ("k", 2), ("q", 2),
    ("k", 3), ("q", 3),
    ("v", 2), ("v", 3),
]


def build_nc():
    nc = bacc.Bacc("TRN2", target_bir_lowering=False, debug=False)

    xkT_d = nc.dram_tensor("xkT", [NSB, 128, NDC * SB], BF16, kind="ExternalInput").ap()
    xqT_d = nc.dram_tensor("xqT", [NSB, 128, NDC * SB], BF16, kind="ExternalInput").ap()
    xvT_d = nc.dram_tensor("xvT", [NSB, 128, NDC * SB], BF16, kind="ExternalInput").ap()
    w_d = nc.dram_tensor("wkqv", [128, 3 * NDC * E], BF16, kind="ExternalInput").ap()
    mask_d = nc.dram_tensor("mask", [128, 128], BF16, kind="ExternalInput").ap()
    ident_d = nc.dram_tensor("ident64", [64, 64], BF16, kind="ExternalInput").ap()
    av_d = nc.dram_tensor("avout", [NSB, 65, SB], BF16, kind="ExternalOutput").ap()

    xd = {"k": xkT_d, "q": xqT_d, "v": xvT_d}

    with tile.TileContext(nc) as tc, ExitStack() as ctx:
        const = ctx.enter_context(tc.tile_pool(name="const", bufs=1))

        # X^T column blocks: one tile per (tensor, sb)
        xsb = {
            t: [const.tile([128, NDC, SB], BF16, name=f"x{t}{s}") for s in range(NSB)]
            for t in "kqv"
        }

        # weights lead the sync queue (tiny, and the first projection needs
        # them); then the 12 X blocks in consumption order; output stores are
        # emitted later on the same ring so they never steal HBM bandwidth
        # from the load stream.
        wall = const.tile([128, 3, NDC, E], BF16, name="wkqv")
        nc.sync.dma_start(out=wall[:], in_=w_d.rearrange("p (t c e) -> p t c e", t=3, e=E))
        w_tiles = {"k": wall[:, 0], "q": wall[:, 1], "v": wall[:, 2]}
        for t, sb in LOAD_ORDER:
            nc.sync.dma_start(
                out=xsb[t][sb][:].rearrange("p c s -> p (c s)"), in_=xd[t][sb]
            )

        # small constants ride the scalar queue, concurrent with the loads
        mask = const.tile([128, 128], BF16, name="mask")
        nc.scalar.dma_start(out=mask[:], in_=mask_d[:])
        ident64 = const.tile([64, 64], BF16, name="ident64")
        nc.scalar.dma_start(out=ident64[:], in_=ident_d[:])

        kT = const.tile([E, S], BF16, name="kT")
        qT = const.tile([E, S], BF16, name="qT")
        vT = const.tile([E, S], BF16, name="vT")
        vaug = const.tile([128, NKT, 65], BF16, name="vaug")
        nc.vector.memset(vaug[:, :, E : E + 1], 1.0)

        # PSUM budget (8 banks): pj 2 + pvt 1 + sc 2x2 + av 1
        pj_pool = ctx.enter_context(tc.tile_pool(name="pj", bufs=2, space="PSUM"))
        sc_pool = ctx.enter_context(tc.tile_pool(name="sc", bufs=2, space="PSUM"))
        av_pool = ctx.enter_context(tc.tile_pool(name="av", bufs=1, space="PSUM"))
        # exp tiles stay live across the qb2/qb3 v-wait; deep pools so the
        # qb3 exp stream never blocks on buffer reuse
        exp_pool = ctx.enter_context(tc.tile_pool(name="exp", bufs=14))
        osb_pool = ctx.enter_context(tc.tile_pool(name="osb", bufs=2))

        # PE warmup: ramp the HAM clock gate while the first load streams
        warm = const.tile([128, SB], BF16, name="warm")
        nc.vector.memset(warm[:], 0.0)
        wpj = av_pool.tile([E, SB], F32, name="avp")
        for _ in range(N_WARMUP):
            nc.tensor.matmul(
                wpj[:], lhsT=warm[:, 0:E], rhs=warm[:], start=True, stop=True
            )

        kqvT = {"k": kT, "q": qT, "v": vT}

        def proj(t, sb):
            # one 8-chunk PSUM accumulation; PSUM -> SBUF cast split across
            # DVE and ACT halves to halve the proj->consumer chain latency
            pj = pj_pool.tile([E, SB], F32, name="pj")
            for dc in range(NDC):
                nc.tensor.matmul(
                    pj[:],
                    lhsT=w_tiles[t][:, dc, :],
                    rhs=xsb[t][sb][:, dc, :],
                    start=(dc == 0),
                    stop=(dc == NDC - 1),
                )
            nc.vector.tensor_copy(kqvT[t][:, sb * SB : (sb + 1) * SB], pj[:])

        def vfill(sb):
            # vaug[:, kt, 0:64] = vT[:, kt-tile].T via PE bf16 transpose;
            # PSUM staging tiles rotate through the proj pool's banks
            for kt in range(4 * sb, 4 * sb + 4):
                pvt = pj_pool.tile([128, E], BF16, name="pvt", bufs=1)
                nc.tensor.transpose(
                    pvt[:], vT[:, kt * 128 : (kt + 1) * 128], ident64[:]
                )
                nc.vector.tensor_copy(vaug[:, kt, 0:E], pvt[:])

        def scores_pair(qb, kt):
            # two off-diagonal k-tiles share one [128,1024] PSUM tile and a
            # single exp: halves ACT instruction count and deepens lookahead
            scp = sc_pool.tile([128, 2 * SB], F32, name="scp")
            for i, k in enumerate((kt, kt + 1)):
                nc.tensor.matmul(
                    scp[:, i * SB : (i + 1) * SB],
                    lhsT=kT[:, k * 128 : (k + 1) * 128],
                    rhs=qT[:, qb * SB : (qb + 1) * SB],
                    start=True,
                    stop=True,
                )
            et = exp_pool.tile([128, 2 * SB], BF16, name="etp")
            nc.scalar.activation(et[:], scp[:], EXP, scale=0.125)
            return {kt: (et[:, 0:SB], 0), kt + 1: (et[:, SB : 2 * SB], 0)}

        def scores_diag(qb, kt):
            j = kt - 4 * qb  # 0..3 within the diagonal band
            c0 = j * 128 if j > 0 else 0
            scp = sc_pool.tile([128, SB], F32, name="scp")
            nc.tensor.matmul(
                scp[:, c0:],
                lhsT=kT[:, kt * 128 : (kt + 1) * 128],
                rhs=qT[:, qb * SB + c0 : (qb + 1) * SB],
                start=True,
                stop=True,
            )
            et = exp_pool.tile([128, SB], BF16, name="etd", bufs=18)
            nc.scalar.activation(et[:, c0:], scp[:, c0:], EXP, scale=0.125)
            nc.vector.tensor_mul(
                et[:, j * 128 : (j + 1) * 128],
                et[:, j * 128 : (j + 1) * 128],
                mask[:],
            )
            return {kt: (et, c0)}

        def av(qb, avp, n_kt, kt, et, c0):
            nc.tensor.matmul(
                avp[:, c0:],
                lhsT=vaug[:, kt, :],
                rhs=et[:, c0:],
                start=(kt == 0),
                stop=(kt == n_kt - 1),
                skip_group_check=True,
            )

        # emission in consumption order (priorities for the scheduler).
        # A(qb, kt) must be emitted AFTER vfill(kt//4): tile deps are
        # program-order based, so a read emitted before its producer's
        # write sees uninitialized memory.
        avps, ets = {}, {}

        def emit_scores(qb, kt_lo, kt_hi):
            kt = kt_lo
            while kt < kt_hi:
                if kt + 1 < 4 * qb:  # both off-diagonal -> pair
                    r = scores_pair(qb, kt)
                    kt += 2
                else:
                    r = scores_diag(qb, kt)
                    kt += 1
                for k, v in r.items():
                    ets[(qb, k)] = v

        def emit_av(qb, kt_lo, kt_hi):
            n_kt = 4 * qb + 4
            for kt in range(kt_lo, kt_hi):
                et, c0 = ets.pop((qb, kt))
                av(qb, avps[qb], n_kt, kt, et, c0)

        def emit_out(qb):
            osb = osb_pool.tile([65, SB], BF16, name="osb")
            nc.vector.tensor_copy(osb[:], avps[qb][:])
            nc.sync.dma_start(out=av_d[qb], in_=osb[:])

        proj("k", 0); proj("q", 0)
        avps[0] = av_pool.tile([65, SB], F32, name="avp")
        emit_scores(0, 0, 4)
        proj("v", 0); vfill(0)
        emit_av(0, 0, 4); emit_out(0)

        proj("k", 1); proj("q", 1)
        avps[1] = av_pool.tile([65, SB], F32, name="avp")
        emit_scores(1, 0, 8)
        emit_av(1, 0, 4)
        proj("v", 1); vfill(1)
        emit_av(1, 4, 8); emit_out(1)

        proj("k", 2); proj("q", 2)
        avps[2] = av_pool.tile([65, SB], F32, name="avp")
        emit_scores(2, 0, 12)
        emit_av(2, 0, 8)

        proj("k", 3); proj("q", 3)
        avps[3] = av_pool.tile([65, SB], F32, name="avp")
        emit_scores(3, 0, 16)
        emit_av(3, 0, 8)

        proj("v", 2); vfill(2)
        emit_av(2, 8, 12); emit_out(2)
        emit_av(3, 8, 12)
        proj("v", 3); vfill(3)
        # split the final output cast: avp3[:, 0:384] is final after
        # A(3,14) (diagonal A's write shrinking column ranges), so most of
        # the cast hides behind the last AV matmul
        emit_av(3, 12, 15)
        osb3 = osb_pool.tile([65, SB], BF16, name="osb")
        nc.vector.tensor_copy(osb3[:, 0:384], avps[3][:, 0:384])
        emit_av(3, 15, 16)
        nc.vector.tensor_copy(osb3[:, 384:], avps[3][:, 384:])
        nc.sync.dma_start(out=av_d[3], in_=osb3[:])

    nc.compile()
    return nc


_NC = None


def _get_nc():
    global _NC
    if _NC is None:
        _NC = build_nc()
    return _NC


def _in_maps(inputs):
    import ml_dtypes

    bf16 = ml_dtypes.bfloat16
    def wprep(w):
        # [1024, 64] -> [128, NDC*E], d = dc*128 + p
        return np.ascontiguousarray(
            np.asarray(w, np.float32).reshape(NDC, 128, E).transpose(1, 0, 2)
        ).reshape(128, NDC * E).astype(bf16)

    wkqv = np.ascontiguousarray(
        np.concatenate(
            [wprep(inputs["K"]), wprep(inputs["Q"]), wprep(inputs["V"])], axis=1
        )
    )
    mask = np.triu(np.ones((128, 128), np.float32)).astype(bf16)
    ident64 = np.eye(64, dtype=np.float32).astype(bf16)
    xk = np.asarray(inputs["inputs_for_keys"], np.float32)
    xq = np.asarray(inputs["inputs_for_queries"], np.float32)
    xv = np.asarray(inputs["inputs_for_values"], np.float32)
    def xprep(x):
        # [S, D] -> X^T [NSB, 128, NDC*SB]: xT[sb, p, dc*SB + s] =
        # x[sb*SB + s, dc*128 + p]
        xT = x.T.astype(bf16)                       # [D, S] = [(dc p), (sb s)]
        xT = xT.reshape(NDC, 128, NSB, SB)
        return np.ascontiguousarray(xT.transpose(2, 1, 0, 3)).reshape(
            NSB, 128, NDC * SB
        )

    maps = []
    for b in range(N_CORES):
        m = {
            "xkT": xprep(xk[b]),
            "xqT": xprep(xq[b]),
            "xvT": xprep(xv[b]),
            "wkqv": wkqv,
            "mask": mask,
            "ident64": ident64,
        }
        maps.append(m)
    return maps


def _post(res):
    out = np.empty((N_CORES, S, E), np.float32)
    for b in range(N_CORES):
        av = np.asarray(res.results[b]["avout"], np.float32)  # [NSB, 65, SB]
        num = av[:, :E, :]                                    # [NSB, 64, SB]
        den = av[:, E : E + 1, :]                             # [NSB, 1, SB]
        o = num / den                                         # [NSB, 64, SB]
        out[b] = o.transpose(0, 2, 1).reshape(S, E)
    return out


def kernel(**inputs):
    from concourse.bass_utils import run_bass_kernel_spmd

    nc = _get_nc()
    res = run_bass_kernel_spmd(nc, _in_maps(inputs), core_ids=list(range(N_CORES)))
    return np.ascontiguousarray(_post(res))


def kernel_profiled(**inputs):
    """Like kernel() but with neuron-profile NTFF capture (dev/test use only)."""
    import types

    from trn_agent_boot.trn_boot import _ntff_profile_via_ctypes

    hook = _ntff_profile_via_ctypes("/opt/axon/libaxon_pjrt.so")
    m = types.ModuleType("antenv.axon_hooks")
    m.get_axon_ntff_profile_hook = lambda: hook
    m.set_axon_ntff_profile_hook = lambda h: None
    sys.modules["antenv.axon_hooks"] = m

    from concourse import bass_utils

    bass_utils.upload_artifacts = lambda tmpdir: tmpdir

    nc = _get_nc()
    res = bass_utils.run_bass_kernel_spmd(
        nc,
        _in_maps(inputs),
        core_ids=list(range(N_CORES)),
        trace=True,
        tmpdir="/tmp/attn_trace",
    )
    return np.ascontiguousarray(_post(res)), res


# revision 19
# speedup vs baseline: 1.0077x; 1.0077x over previous
"""Self-contained Trainium2 Bass kernel for nn_AttentionHead_89687507076307.

Problem: single-head causal attention, B=8, S=2048, D_IN=1024, D_OUT=64, fp32.
Sharding: data-parallel over batch -- each of the 8 NeuronCores computes one
batch element end to end; no collectives.

Host marshaling (part of input sharding): X tensors are transposed to
[D, S] layout and cast to bf16 per core; weights are cast to bf16 and split
into 128-row d-chunks.  The device then runs a pure-bf16 matmul pipeline
(PE native rate) with no on-device transposes of X:

  kT/qT/vT [64,S]: W-chunk stationary, X^T pumped  (contract d, PSUM accum)
  vaug  [k,65]   : PE transpose of vT tiles + ones col (natural [S,64]+sums)
  scoresT [k,q]  : kT-tile stationary, qT pumped    (contract e=64); two
                   off-diagonal k-tiles share one [128,1024] PSUM pair
  expT           : ACT exp(0.125 * scores) -> bf16 over the 1024-wide pair
                   (halves ACT instruction count); causal quarter-mask on
                   diagonal tiles (DVE); fully-masked columns skipped
  av [65, q]     : vaug stationary, expT pumped     (contract k; row 64=sums)

The load stream is ordered wkqv k0 q0 v0 k1 q1 v1 k2 q2 k3 q3 v2 v3 so the
last query block's scores/exp chain starts early and only the cheap
v-gated AV work trails the final DMA; a PE warmup bridge holds the HAM
clock gate at full rate until the first data lands.  Device emits
unnormalized av tiles [4, 65, 512] bf16; the host divides by the sums row
and transposes back to [S, 64] during the gather/unshard step.
"""
import sys

for _p in ("/opt/trn_rl_repo",):
    if _p not in sys.path:
        sys.path.append(_p)

from contextlib import ExitStack

import numpy as np

import concourse.bass as bass
import concourse.mybir as mybir
import concourse.tile as tile
from concourse import bacc

B, S, D, E = 8, 2048, 1024, 64
SB = 512               # q block size
NSB = S // SB          # 4
NKT = S // 128         # 16 k-tiles
NDC = D // 128         # 8 d-chunks
F32 = mybir.dt.float32
BF16 = mybir.dt.bfloat16
EXP = mybir.ActivationFunctionType.Exp
N_CORES = 8
N_WARMUP = 15

# per-sb load order of (tensor, sb) blocks; projections/attention consume in
# this order too.  k3/q3 pulled ahead of v2/v3 so the tail after the last
# load is only v-projection + diagonal AV.
LOAD_ORDER = [
    ("k", 0), ("q", 0), ("v", 0),
    ("k", 1), ("q", 1), ("v", 1),
    ("k", 2), ("q", 2),
    ("k", 3), ("q", 3),
    ("v", 2), ("v", 3),
]


def build_nc():
    nc = bacc.Bacc("TRN2", target_bir_lowering=False, debug=False)

    xkT_d = nc.dram_tensor("xkT", [NSB, 128, NDC * SB], BF16, kind="ExternalInput").ap()
    xqT_d = nc.dram_tensor("xqT", [NSB, 128, NDC * SB], BF16, kind="ExternalInput").ap()
    xvT_d = nc.dram_tensor("xvT", [NSB, 128, NDC * SB], BF16, kind="ExternalInput").ap()
    w_d = nc.dram_tensor("wkqv", [128, 3 * NDC * E], BF16, kind="ExternalInput").ap()
    mask_d = nc.dram_tensor("mask", [128, 128], BF16, kind="ExternalInput").ap()
    ident_d = nc.dram_tensor("ident64", [64, 64], BF16, kind="ExternalInput").ap()
    av_d = nc.dram_tensor("avout", [NSB, 65, SB], BF16, kind="ExternalOutput").ap()

    xd = {"k": xkT_d, "q": xqT_d, "v": xvT_d}

    with tile.TileContext(nc) as tc, ExitStack() as ctx:
        const = ctx.enter_context(tc.tile_pool(name="const", bufs=1))

        # X^T column blocks: one tile per (tensor, sb)
        xsb = {
            t: [const.tile([128, NDC, SB], BF16, name=f"x{t}{s}") for s in range(NSB)]
            for t in "kqv"
        }

        # weights lead the sync queue (tiny, and the first projection needs
        # them); then the 12 X blocks in consumption order; output stores are
        # emitted later on the same ring so they never steal HBM bandwidth
        # from the load stream.
        wall = const.tile([128, 3, NDC, E], BF16, name="wkqv")
        nc.sync.dma_start(out=wall[:], in_=w_d.rearrange("p (t c e) -> p t c e", t=3, e=E))
        w_tiles = {"k": wall[:, 0], "q": wall[:, 1], "v": wall[:, 2]}
        for t, sb in LOAD_ORDER:
            nc.sync.dma_start(
                out=xsb[t][sb][:].rearrange("p c s -> p (c s)"), in_=xd[t][sb]
            )

        # small constants ride the scalar queue, concurrent with the loads
        mask = const.tile([128, 128], BF16, name="mask")
        nc.scalar.dma_start(out=mask[:], in_=mask_d[:])
        ident64 = const.tile([64, 64], BF16, name="ident64")
        nc.scalar.dma_start(out=ident64[:], in_=ident_d[:])

        kT = const.tile([E, S], BF16, name="kT")
        qT = const.tile([E, S], BF16, name="qT")
        vT = const.tile([E, S], BF16, name="vT")
        vaug = const.tile([128, NKT, 65], BF16, name="vaug")
        nc.vector.memset(vaug[:, :, E : E + 1], 1.0)

        # PSUM budget (8 banks): pj 2 + pvt 1 + sc 2x2 + av 1
        pj_pool = ctx.enter_context(tc.tile_pool(name="pj", bufs=2, space="PSUM"))
        sc_pool = ctx.enter_context(tc.tile_pool(name="sc", bufs=2, space="PSUM"))
        av_pool = ctx.enter_context(tc.tile_pool(name="av", bufs=1, space="PSUM"))
        # exp tiles stay live across the qb2/qb3 v-wait; deep pools so the
        # qb3 exp stream never blocks on buffer reuse
        exp_pool = ctx.enter_context(tc.tile_pool(name="exp", bufs=14))
        osb_pool = ctx.enter_context(tc.tile_pool(name="osb", bufs=2))

        # PE warmup: ramp the HAM clock gate while the first load streams
        warm = const.tile([128, SB], BF16, name="warm")
        nc.vector.memset(warm[:], 0.0)
        wpj = av_pool.tile([E, SB], F32, name="avp")
        for _ in range(N_WARMUP):
            nc.tensor.matmul(
                wpj[:], lhsT=warm[:, 0:E], rhs=warm[:], start=True, stop=True
            )

        kqvT = {"k": kT, "q": qT, "v": vT}

        def pe_fill(n):
            # dependency-free filler matmuls: absorb the Pq->cast->S chain
            # latency so the PE (the pacing engine) never idles there
            fl = pj_pool.tile([E, E], F32, name="pvt", bufs=1)
            for _ in range(n):
                nc.tensor.matmul(
                    fl[:], lhsT=warm[:, 0:E], rhs=warm[:, 0:E],
                    start=True, stop=True,
                )

        def proj(t, sb):
            # one 8-chunk PSUM accumulation; PSUM -> SBUF cast split across
            # DVE and ACT halves to halve the proj->consumer chain latency
            pj = pj_pool.tile([E, SB], F32, name="pj")
            for dc in range(NDC):
                nc.tensor.matmul(
                    pj[:],
                    lhsT=w_tiles[t][:, dc, :],
                    rhs=xsb[t][sb][:, dc, :],
                    start=(dc == 0),
                    stop=(dc == NDC - 1),
                )
            nc.vector.tensor_copy(kqvT[t][:, sb * SB : (sb + 1) * SB], pj[:])

        def vfill(sb):
            # vaug[:, kt, 0:64] = vT[:, kt-tile].T via PE bf16 transpose;
            # PSUM staging tiles rotate through the proj pool's banks
            for kt in range(4 * sb, 4 * sb + 4):
                pvt = pj_pool.tile([128, E], BF16, name="pvt", bufs=1)
                nc.tensor.transpose(
                    pvt[:], vT[:, kt * 128 : (kt + 1) * 128], ident64[:]
                )
                nc.vector.tensor_copy(vaug[:, kt, 0:E], pvt[:])

        def scores_pair(qb, kt):
            # two off-diagonal k-tiles share one [128,1024] PSUM tile and a
            # single exp: halves ACT instruction count and deepens lookahead
            scp = sc_pool.tile([128, 2 * SB], F32, name="scp")
            for i, k in enumerate((kt, kt + 1)):
                nc.tensor.matmul(
                    scp[:, i * SB : (i + 1) * SB],
                    lhsT=kT[:, k * 128 : (k + 1) * 128],
                    rhs=qT[:, qb * SB : (qb + 1) * SB],
                    start=True,
                    stop=True,
                )
            et = exp_pool.tile([128, 2 * SB], BF16, name="etp")
            nc.scalar.activation(et[:], scp[:], EXP, scale=0.125)
            return {kt: (et[:, 0:SB], 0), kt + 1: (et[:, SB : 2 * SB], 0)}

        def scores_diag(qb, kt):
            j = kt - 4 * qb  # 0..3 within the diagonal band
            c0 = j * 128 if j > 0 else 0
            scp = sc_pool.tile([128, SB], F32, name="scp")
            nc.tensor.matmul(
                scp[:, c0:],
                lhsT=kT[:, kt * 128 : (kt + 1) * 128],
                rhs=qT[:, qb * SB + c0 : (qb + 1) * SB],
                start=True,
                stop=True,
            )
            et = exp_pool.tile([128, SB], BF16, name="etd", bufs=18)
            nc.scalar.activation(et[:, c0:], scp[:, c0:], EXP, scale=0.125)
            nc.vector.tensor_mul(
                et[:, j * 128 : (j + 1) * 128],
                et[:, j * 128 : (j + 1) * 128],
                mask[:],
            )
            return {kt: (et, c0)}

        def av(qb, avp, n_kt, kt, et, c0):
            nc.tensor.matmul(
                avp[:, c0:],
                lhsT=vaug[:, kt, :],
                rhs=et[:, c0:],
                start=(kt == 0),
                stop=(kt == n_kt - 1),
                skip_group_check=True,
            )

        # emission in consumption order (priorities for the scheduler).
        # A(qb, kt) must be emitted AFTER vfill(kt//4): tile deps are
        # program-order based, so a read emitted before its producer's
        # write sees uninitialized memory.
        avps, ets = {}, {}

        def emit_scores(qb, kt_lo, kt_hi):
            kt = kt_lo
            while kt < kt_hi:
                if kt + 1 < 4 * qb:  # both off-diagonal -> pair
                    r = scores_pair(qb, kt)
                    kt += 2
                else:
                    r = scores_diag(qb, kt)
                    kt += 1
                for k, v in r.items():
                    ets[(qb, k)] = v

        def emit_av(qb, kt_lo, kt_hi):
            n_kt = 4 * qb + 4
            for kt in range(kt_lo, kt_hi):
                et, c0 = ets.pop((qb, kt))
                av(qb, avps[qb], n_kt, kt, et, c0)

        def emit_out(qb):
            osb = osb_pool.tile([65, SB], BF16, name="osb")
            nc.vector.tensor_copy(osb[:], avps[qb][:])
            nc.sync.dma_start(out=av_d[qb], in_=osb[:])

        proj("k", 0); proj("q", 0)
        pe_fill(3)
        avps[0] = av_pool.tile([65, SB], F32, name="avp")
        emit_scores(0, 0, 4)
        proj("v", 0); vfill(0)
        emit_av(0, 0, 4); emit_out(0)

        proj("k", 1); proj("q", 1)
        pe_fill(3)
        avps[1] = av_pool.tile([65, SB], F32, name="avp")
        emit_scores(1, 0, 8)
        emit_av(1, 0, 4)
        proj("v", 1); vfill(1)
        emit_av(1, 4, 8); emit_out(1)

        proj("k", 2); proj("q", 2)
        pe_fill(3)
        avps[2] = av_pool.tile([65, SB], F32, name="avp")
        emit_scores(2, 0, 12)
        emit_av(2, 0, 8)

        proj("k", 3); proj("q", 3)
        pe_fill(3)
        avps[3] = av_pool.tile([65, SB], F32, name="avp")
        emit_scores(3, 0, 16)
        emit_av(3, 0, 8)

        proj("v", 2); vfill(2)
        emit_av(2, 8, 12); emit_out(2)
        emit_av(3, 8, 12)
        proj("v", 3); vfill(3)
        # split the final output cast: avp3[:, 0:384] is final after
        # A(3,14) (diagonal A's write shrinking column ranges), so most of
        # the cast hides behind the last AV matmul
        emit_av(3, 12, 15)
        osb3 = osb_pool.tile([65, SB], BF16, name="osb")
        nc.vector.tensor_copy(osb3[:, 0:384], avps[3][:, 0:384])
        emit_av(3, 15, 16)
        nc.vector.tensor_copy(osb3[:, 384:], avps[3][:, 384:])
        nc.sync.dma_start(out=av_d[3], in_=osb3[:])

    nc.compile()
    return nc


_NC = None


def _get_nc():
    global _NC
    if _NC is None:
        _NC = build_nc()
    return _NC


def _in_maps(inputs):
    import ml_dtypes

    bf16 = ml_dtypes.bfloat16
    def wprep(w):
        # [1024, 64] -> [128, NDC*E], d = dc*128 + p
        return np.ascontiguousarray(
            np.asarray(w, np.float32).reshape(NDC, 128, E).transpose(1, 0, 2)
        ).reshape(128, NDC * E).astype(bf16)

    wkqv = np.ascontiguousarray(
        np.concatenate(
            [wprep(inputs["K"]), wprep(inputs["Q"]), wprep(inputs["V"])], axis=1
        )
    )
    mask = np.triu(np.ones((128, 128), np.float32)).astype(bf16)
    ident64 = np.eye(64, dtype=np.float32).astype(bf16)
    xk = np.asarray(inputs["inputs_for_keys"], np.float32)
    xq = np.asarray(inputs["inputs_for_queries"], np.float32)
    xv = np.asarray(inputs["inputs_for_values"], np.float32)
    def xprep(x):
        # [S, D] -> X^T [NSB, 128, NDC*SB]: xT[sb, p, dc*SB + s] =
        # x[sb*SB + s, dc*128 + p]
        xT = x.T.astype(bf16)                       # [D, S] = [(dc p), (sb s)]
        xT = xT.reshape(NDC, 128, NSB, SB)
        return np.ascontiguousarray(xT.transpose(2, 1, 0, 3)).reshape(
            NSB, 128, NDC * SB
        )

    maps = []
    for b in range(N_CORES):
        m = {
            "xkT": xprep(xk[b]),
            "xqT": xprep(xq[b]),
            "xvT": xprep(xv[b]),
            "wkqv": wkqv,
            "mask": mask,
            "ident64": ident64,
        }
        maps.append(m)
    return maps


def _post(res):
    out = np.empty((N_CORES, S, E), np.float32)
    for b in range(N_CORES):
        av = np.asarray(res.results[b]["avout"], np.float32)  # [NSB, 65, SB]
        num = av[:, :E, :]                                    # [NSB, 64, SB]
        den = av[:, E : E + 1, :]                             # [NSB, 1, SB]
        o = num / den                                         # [NSB, 64, SB]
        out[b] = o.transpose(0, 2, 1).reshape(S, E)
    return out


def kernel(**inputs):
    from concourse.bass_utils import run_bass_kernel_spmd

    nc = _get_nc()
    res = run_bass_kernel_spmd(nc, _in_maps(inputs), core_ids=list(range(N_CORES)))
    return np.ascontiguousarray(_post(res))


def kernel_profiled(**inputs):
    """Like kernel() but with neuron-profile NTFF capture (dev/test use only)."""
    import types

    from trn_agent_boot.trn_boot import _ntff_profile_via_ctypes

    hook = _ntff_profile_via_ctypes("/opt/axon/libaxon_pjrt.so")
    m = types.ModuleType("antenv.axon_hooks")
    m.get_axon_ntff_profile_hook = lambda: hook
    m.set_axon_ntff_profile_hook = lambda h: None
    sys.modules["antenv.axon_hooks"] = m

    from concourse import bass_utils

    bass_utils.upload_artifacts = lambda tmpdir: tmpdir

    nc = _get_nc()
    res = bass_utils.run_bass_kernel_spmd(
        nc,
        _in_maps(inputs),
        core_ids=list(range(N_CORES)),
        trace=True,
        tmpdir="/tmp/attn_trace",
    )
    return np.ascontiguousarray(_post(res)), res
